# revision 1
# baseline (speedup 1.0000x reference)
"""Bi-attention kernel for Trainium2 (8 NeuronCores, data-parallel over batch).

Per-core computation (B=1 slice, Lc=512, Lq=64, D=256):
  score[i,j] = c_i.w_c + q_j.w_q + sum_d c[i,d] q[j,d] w_p[d] + b - 1e30*(1-mask[j])
  h = softmax_j(score);  U[i] = sum_j h[i,j] * (q_j.w_mem)
  u = softmax_i(max_j score);  H = sum_i u[i] * (c_i.w_in)
  G[i] = [ctx1[i], U[i], ctx1[i]*U[i], U[i]*H]

Sharding/layout choice: data-parallel over batch, one batch element per
core. Per-core inputs are laid out for the tensor engine during sharding:
context and question are shipped transposed (contraction dim D on
partitions), and all small params (att_w split, w_in, w_mem, att_b, qT)
ride in one packed [128, 139] tensor -> 7 DMAs total per core.

Device mapping:
  - score matmuls: per 128-row chunk, PSUM [128, 66] = scores | c.w_c |
    c.w_in; the per-column constants (q_j.w_q + b + mask) are added by a
    K=1 ones-row matmul into the same accumulation group.
  - row softmax: scores are O(10) so exp needs no max shift; ACT Exp with
    accum_out yields numerator-free denominator; the row max (needed for
    the second softmax's input m) runs in parallel on DVE.
  - softmax over i (partition dim): exp of per-chunk maxes, partition sums
    via ones-column matmul, scalar division, broadcast via ones-row matmul.
"""

import sys

for _p in ("/opt/trn_rl_repo", "/root/.axon_site/_ro/trn_rl_repo"):
    if _p not in sys.path:
        sys.path.append(_p)

import numpy as np

import concourse.bacc as bacc
import concourse.bass as bass
import concourse.tile as tile
from concourse import mybir
from concourse.bass_utils import run_bass_kernel_spmd

B, LC, LQ, D = 8, 512, 64, 256
NEG_BIG = 1e30
NCHUNK = LC // 128  # 4 chunks of 128 context rows
KD = D // 128  # 2 contraction chunks
F32 = mybir.dt.float32
I32 = mybir.dt.int32
AF = mybir.ActivationFunctionType
ALU = mybir.AluOpType
AX = mybir.AxisListType

# params tensor column layout (packed on host)
PC_WC = 0  # cols 0:2    w_c chunks
PC_WQ = 2  # cols 2:4    w_q chunks
PC_WP = 4  # cols 4:6    w_p chunks
PC_WIN = 6  # cols 6:8    w_in chunks
PC_WMEM = 8  # cols 8:10   w_mem chunks
PC_B = 10  # col 10      att_b at row 0
PC_QT = 11  # cols 11:139 qT chunks: [11+64k : 75+64k] = question.T chunk k
NPC = 11 + KD * LQ


def build_nc():
    nc = bacc.Bacc("TRN2", target_bir_lowering=False, debug=False)

    ctxt_d = nc.dram_tensor("contextT", [KD, 128, LC], F32, kind="ExternalInput")
    mask_d = nc.dram_tensor("mask", [1, LQ], I32, kind="ExternalInput")
    par_d = nc.dram_tensor("params", [128, NPC], F32, kind="ExternalInput")
    g_d = nc.dram_tensor("G", [LC, 4], F32, kind="ExternalOutput")

    with tile.TileContext(nc) as tc:
        with (
            tc.tile_pool(name="singles", bufs=1) as singles,
            tc.tile_pool(name="work", bufs=2) as work,
            tc.tile_pool(name="ps_sc", bufs=4, space="PSUM") as ps_sc,
            tc.tile_pool(name="ps_misc", bufs=1, space="PSUM") as ps_misc,
        ):
            # ---- params first on SP; context halves on Pool + SP ----
            par = singles.tile([128, NPC], F32)
            nc.sync.dma_start(out=par, in_=par_d[:, :])
            cT = singles.tile([128, KD, LC], F32)
            half = LC // 2
            nc.gpsimd.dma_start(out=cT[:, 0, 0:half], in_=ctxt_d[0, :, 0:half])
            nc.sync.dma_start(out=cT[:, 1, 0:half], in_=ctxt_d[1, :, 0:half])
            nc.gpsimd.dma_start(
                out=cT[:, 0, half:LC], in_=ctxt_d[0, :, half:LC]
            )
            nc.sync.dma_start(out=cT[:, 1, half:LC], in_=ctxt_d[1, :, half:LC])
            ones_row = singles.tile([1, 128], F32)
            nc.gpsimd.memset(ones_row, 1.0)
            ones_col = singles.tile([128, 1], F32)
            nc.gpsimd.memset(ones_col, 1.0)
            warm = singles.tile([1, 1], F32)
            nc.scalar.activation(warm, ones_row[0:1, 0:1], AF.Exp)
            mask_i = singles.tile([1, LQ], I32)
            nc.gpsimd.dma_start(out=mask_i, in_=mask_d[:, :])

            def qt(k):
                return par[:, PC_QT + LQ * k : PC_QT + LQ * (k + 1)]

            # rhsA_k [128, 66]: cols 0:64 = w_p * qT, col 64 = w_c, col 65 = w_in
            rhsA = []
            for k in range(KD):
                rhsA_k = singles.tile(
                    [128, LQ + 2], F32, tag=f"rhsA{k}", name=f"rhsA{k}"
                )
                nc.vector.tensor_scalar_mul(
                    rhsA_k[:, 0:LQ], qt(k), par[:, PC_WP + k : PC_WP + k + 1]
                )
                nc.vector.tensor_copy(
                    rhsA_k[:, LQ : LQ + 1], par[:, PC_WC + k : PC_WC + k + 1]
                )
                nc.vector.tensor_copy(
                    rhsA_k[:, LQ + 1 : LQ + 2], par[:, PC_WIN + k : PC_WIN + k + 1]
                )
                rhsA.append(rhsA_k)

            # ---- sq/q1 rows: [1, 64] each = w.T @ qT ----
            sq_ps = ps_misc.tile([1, LQ], F32, tag="early", name="sq_ps")
            for k in range(KD):
                nc.tensor.matmul(
                    sq_ps,
                    par[:, PC_WQ + k : PC_WQ + k + 1],
                    qt(k),
                    start=(k == 0),
                    stop=(k == KD - 1),
                )
            q1_ps = ps_misc.tile([1, LQ], F32, tag="earlyb", name="q1_ps")
            for k in range(KD):
                nc.tensor.matmul(
                    q1_ps,
                    par[:, PC_WMEM + k : PC_WMEM + k + 1],
                    qt(k),
                    start=(k == 0),
                    stop=(k == KD - 1),
                )

            # row_vec [1, 66]: cols j = sq[j] + b - 1e30*(1-mask[j]); 64,65 = 0
            row_vec = singles.tile([1, LQ + 2], F32)
            nc.gpsimd.memset(row_vec, 0.0)
            maskf = singles.tile([1, LQ], F32)
            nc.gpsimd.tensor_copy(maskf, mask_i)
            maskt = singles.tile([1, LQ], F32)
            nc.gpsimd.tensor_scalar(
                maskt, maskf, NEG_BIG, -NEG_BIG, op0=ALU.mult, op1=ALU.add
            )
            sqb = singles.tile([1, LQ], F32)
            nc.vector.tensor_scalar_add(sqb, sq_ps, par[0:1, PC_B : PC_B + 1])
            nc.vector.tensor_add(row_vec[0:1, 0:LQ], maskt, sqb)

            # q1 broadcast to all partitions: [128, 64] in PSUM
            q1row = singles.tile([1, LQ], F32)
            nc.vector.tensor_copy(q1row, q1_ps)
            q1bc_ps = ps_misc.tile([128, LQ], F32, tag="q1bc")
            nc.tensor.matmul(q1bc_ps, ones_row, q1row, start=True, stop=True)

            # ---- per-chunk: score matmuls + row softmax + U ----
            m_all = singles.tile([128, NCHUNK], F32)
            ctx1_all = singles.tile([128, NCHUNK], F32)
            g_all = singles.tile([128, NCHUNK, 4], F32)
            for c in range(NCHUNK):
                sc_ps = ps_sc.tile([128, LQ + 2], F32, tag="score", name=f"sc{c}")
                for k in range(KD):
                    nc.tensor.matmul(
                        sc_ps,
                        cT[:, k, 128 * c : 128 * (c + 1)],
                        rhsA[k],
                        start=(k == 0),
                        stop=False,
                    )
                nc.tensor.matmul(sc_ps, ones_row, row_vec, start=False, stop=True)

                t_ap = sc_ps[:, 0:LQ]
                rmax = work.tile([128, 1], F32, tag="rmax")
                nc.vector.tensor_reduce(rmax, t_ap, AX.X, ALU.max)
                nc.vector.tensor_add(m_all[:, c : c + 1], sc_ps[:, LQ : LQ + 1], rmax)
                # scores are O(10): exp is fp32-safe without max shift
                e_t = work.tile([128, LQ], F32, tag="e")
                den = work.tile([128, 1], F32, tag="den")
                nc.scalar.activation(e_t, t_ap, AF.Exp, accum_out=den)
                prod = work.tile([128, LQ], F32, tag="prod")
                num = work.tile([128, 1], F32, tag="num")
                nc.vector.tensor_mul(prod, e_t, q1bc_ps)
                nc.vector.reduce_sum(num, prod, axis=AX.X, op=ALU.add)
                rden = work.tile([128, 1], F32, tag="rden")
                nc.vector.reciprocal(rden, den)
                nc.vector.tensor_mul(g_all[:, c, 1:2], num, rden)  # U
                nc.vector.tensor_copy(ctx1_all[:, c : c + 1], sc_ps[:, LQ + 1 : LQ + 2])
                nc.vector.tensor_copy(g_all[:, c, 0:1], sc_ps[:, LQ + 1 : LQ + 2])
                nc.gpsimd.tensor_mul(
                    g_all[:, c, 2:3], ctx1_all[:, c : c + 1], g_all[:, c, 1:2]
                )

            # ---- u_aware softmax over i (512 values) + H ----
            exu = singles.tile([128, 2 * NCHUNK], F32)
            nc.scalar.activation(exu[:, 0:NCHUNK], m_all, AF.Exp)
            nc.gpsimd.tensor_mul(
                exu[:, NCHUNK : 2 * NCHUNK], exu[:, 0:NCHUNK], ctx1_all
            )
            hsum_ps = ps_misc.tile([1, 2 * NCHUNK], F32, tag="late")
            nc.tensor.matmul(hsum_ps, ones_col, exu, start=True, stop=True)
            dn = singles.tile([1, 2], F32)
            nc.vector.tensor_reduce(
                dn.rearrange("o (c f) -> o c f", c=2),
                hsum_ps[0:1, :].rearrange("o (c f) -> o c f", c=2),
                AX.X,
                ALU.add,
            )
            rden_u = singles.tile([1, 1], F32)
            nc.vector.reciprocal(rden_u, dn[0:1, 0:1])
            h_sb = singles.tile([1, 1], F32)
            nc.vector.tensor_mul(h_sb, dn[0:1, 1:2], rden_u)
            hbc_ps = ps_misc.tile([128, 1], F32, tag="late", name="hbc_ps")
            nc.tensor.matmul(hbc_ps, ones_row, h_sb, start=True, stop=True)

            nc.vector.tensor_scalar_mul(
                g_all[:, :, 3:4].rearrange("q c o -> q (c o)"),
                g_all[:, :, 1:2].rearrange("q c o -> q (c o)"),
                hbc_ps,
            )
            nc.sync.dma_start(
                out=g_d.rearrange("(c p) g -> p c g", p=128), in_=g_all
            )

    nc.finalize()
    return nc


_NC = None


def _get_nc():
    global _NC
    if _NC is None:
        _NC = build_nc()
    return _NC


def pack_params(att_w, att_b, w_in, w_mem, question_b):
    par = np.zeros((128, NPC), np.float32)
    par[:, PC_WC : PC_WC + 2] = att_w[0:256].reshape(2, 128).T
    par[:, PC_WQ : PC_WQ + 2] = att_w[256:512].reshape(2, 128).T
    par[:, PC_WP : PC_WP + 2] = att_w[512:768].reshape(2, 128).T
    par[:, PC_WIN : PC_WIN + 2] = w_in.reshape(2, 128).T
    par[:, PC_WMEM : PC_WMEM + 2] = w_mem.reshape(2, 128).T
    par[0, PC_B] = att_b[0]
    qt = question_b.T.reshape(KD, 128, LQ)  # [d, j] split into chunks
    for k in range(KD):
        par[:, PC_QT + LQ * k : PC_QT + LQ * (k + 1)] = qt[k]
    return par


def make_in_maps(context, question, mask, att_w, att_b, w_in, w_mem):
    context = np.asarray(context, np.float32)
    question = np.asarray(question, np.float32)
    mask = np.asarray(mask, np.int32)
    att_w = np.asarray(att_w, np.float32)
    att_b = np.asarray(att_b, np.float32)
    w_in = np.asarray(w_in, np.float32)
    w_mem = np.asarray(w_mem, np.float32)
    maps = []
    for b in range(B):
        ctxt = np.ascontiguousarray(context[b].T).reshape(KD, 128, LC)
        maps.append(
            {
                "contextT": ctxt,
                "mask": mask[b][None, :],
                "params": pack_params(att_w, att_b, w_in, w_mem, question[b]),
            }
        )
    return maps


def kernel(context, question, mask, att_w, att_b, w_in, w_mem):
    nc = _get_nc()
    in_maps = make_in_maps(context, question, mask, att_w, att_b, w_in, w_mem)
    res = run_bass_kernel_spmd(nc, in_maps, core_ids=list(range(B)))
    return np.stack([res.results[c]["G"] for c in range(B)], axis=0)



# revision 7
# speedup vs baseline: 1.0456x; 1.0456x over previous
"""Bi-attention kernel for Trainium2 (8 NeuronCores, data-parallel over batch).

Per-core computation (B=1 slice, Lc=512, Lq=64, D=256):
  score[i,j] = c_i.w_c + q_j.w_q + sum_d c[i,d] q[j,d] w_p[d] + b - 1e30*(1-mask[j])
  h = softmax_j(score);  U[i] = sum_j h[i,j] * (q_j.w_mem)
  u = softmax_i(max_j score);  H = sum_i u[i] * (c_i.w_in)
  G[i] = [ctx1[i], U[i], ctx1[i]*U[i], U[i]*H]

Layout: i (context rows) on partitions, 4 chunks of 128. All matmuls in
bf16 (1 cycle/row vs 4 for fp32); softmax math in fp32/bf16 mix.

Key scheduling ideas (v1 CoreSim cost model):
  - 3 input DMAs issued at t~100 on three different queues (ACT: params,
    Pool: cT k=0, DVE: cT k=1) so all land by ~2500 and nothing queues.
  - One PSUM tile [128, 4, 66] holds all 4 chunks' scores | sc | ctx1.
    Per chunk: 2 bf16 cT matmuls + 1 ones-row matmul (adds sq_j + b + mask
    per column); sq itself comes from a 3-matmul group that folds the
    mask/bias row in PSUM, so only one PSUM->SBUF copy is on the path.
  - ONE fused exp over the whole tile (incl. sc col -> exp(sc) for the
    u-softmax: max_j exp(s) == exp(max_j s), so no second ACT pass).
  - Reductions on DVE (only engine with free-axis reduce): rmax, den,
    and 4 TensorTensorReduce for num (mul+reduce fused). u-softmax sums
    via TTR into w12, partition-summed by a free-size-1 PE matmul (~0ns).
  - Pool (no PSUM access on TRN2) handles SBUF-only elementwise: rhsA0
    prep, G assembly. Free-size<=4 ops cost ~0-3ns.
"""

import sys

for _p in ("/opt/trn_rl_repo", "/root/.axon_site/_ro/trn_rl_repo"):
    if _p not in sys.path:
        sys.path.append(_p)

import ml_dtypes
import numpy as np

import concourse.bacc as bacc
import concourse.bass as bass
import concourse.tile as tile
from concourse import mybir
from concourse.bass_utils import run_bass_kernel_spmd

B, LC, LQ, D = 8, 512, 64, 256
NEG_BIG = 1e30
NCHUNK = LC // 128  # 4 chunks of 128 context rows
KD = D // 128  # 2 contraction chunks
F32 = mybir.dt.float32
BF16 = mybir.dt.bfloat16
NPBF = np.dtype(ml_dtypes.bfloat16)
AF = mybir.ActivationFunctionType
ALU = mybir.AluOpType
AX = mybir.AxisListType

# params tensor column layout (bf16, packed on host)
PC_QT = 0  # cols 0:128    qT chunks: [64k : 64(k+1)] = question.T chunk k
PC_WQ = 128  # cols 128:130  w_q chunks
PC_WM = 130  # cols 130:132  w_mem chunks
PC_WP = 132  # cols 132:134  w_p chunks
PC_WC = 134  # cols 134:136  w_c chunks
PC_WIN = 136  # cols 136:138  w_in chunks
PC_MB = 138  # cols 138:202  row 0 = att_b - 1e30*(1-mask)
NPC = PC_MB + LQ


def build_nc():
    nc = bacc.Bacc("TRN2", target_bir_lowering=False, debug=False)

    ct0_d = nc.dram_tensor("cT0", [128, LC], BF16, kind="ExternalInput")
    ct1_d = nc.dram_tensor("cT1", [128, LC], BF16, kind="ExternalInput")
    par_d = nc.dram_tensor("par", [128, NPC], BF16, kind="ExternalInput")
    g_d = nc.dram_tensor("G", [LC, 4], F32, kind="ExternalOutput")

    with tile.TileContext(nc) as tc:
        with (
            tc.tile_pool(name="singles", bufs=1) as singles,
            tc.tile_pool(name="ps_sc", bufs=1, space="PSUM") as ps_sc,
            tc.tile_pool(name="ps_misc", bufs=1, space="PSUM") as ps_misc,
        ):
            # ---- input DMAs first, one per queue ----
            par = singles.tile([128, NPC], BF16)
            nc.scalar.dma_start(out=par, in_=par_d[:, :])
            cT0 = singles.tile([128, LC], BF16)
            nc.gpsimd.dma_start(out=cT0, in_=ct0_d[:, :])
            cT1 = singles.tile([128, LC], BF16)
            nc.sync.dma_start(out=cT1, in_=ct1_d[:, :])

            # ---- constants ----
            ones_bf = singles.tile([1, 128], BF16)
            nc.gpsimd.memset(ones_bf, 1.0)
            ones_f = singles.tile([1, 128], F32)
            nc.gpsimd.memset(ones_f, 1.0)
            ones_col = singles.tile([128, 1], F32)
            nc.gpsimd.memset(ones_col, 1.0)
            row_vec = singles.tile([1, LQ + 2], BF16)
            nc.gpsimd.memset(row_vec, 0.0)
            warm = singles.tile([1, 1], F32)
            nc.scalar.activation(warm, ones_f[0:1, 0:1], AF.Exp)

            def qt(k):
                return par[:, PC_QT + LQ * k : PC_QT + LQ * (k + 1)]

            # rhsA_k [128, 66]: cols 0:64 = w_p * qT, col 64 = w_c, col 65 = w_in
            rhsA = []
            for k, eng in ((0, nc.gpsimd), (1, nc.vector)):
                r = singles.tile([128, LQ + 2], BF16, tag=f"rhsA{k}", name=f"rhsA{k}")
                wp_f = singles.tile([128, 1], F32, tag=f"wpf{k}", name=f"wpf{k}")
                eng.tensor_copy(wp_f, par[:, PC_WP + k : PC_WP + k + 1])
                eng.tensor_scalar_mul(r[:, 0:LQ], qt(k), wp_f)
                eng.tensor_copy(r[:, LQ : LQ + 1], par[:, PC_WC + k : PC_WC + k + 1])
                eng.tensor_copy(
                    r[:, LQ + 1 : LQ + 2], par[:, PC_WIN + k : PC_WIN + k + 1]
                )
                rhsA.append(r)

            # ---- sq group: sq_j + b - 1e30*(1-mask_j), folded in PSUM ----
            sq_ps = ps_misc.tile([1, LQ], F32, tag="sq", name="sq_ps")
            for k in range(KD):
                nc.tensor.matmul(
                    sq_ps,
                    par[:, PC_WQ + k : PC_WQ + k + 1],
                    qt(k),
                    start=(k == 0),
                    stop=False,
                )
            nc.tensor.matmul(
                sq_ps,
                ones_bf[0:1, 0:1],
                par[0:1, PC_MB : PC_MB + LQ],
                start=False,
                stop=True,
            )
            # q1 group: q1_j = q_j . w_mem
            q1_ps = ps_misc.tile([1, LQ], F32, tag="q1", name="q1_ps")
            for k in range(KD):
                nc.tensor.matmul(
                    q1_ps,
                    par[:, PC_WM + k : PC_WM + k + 1],
                    qt(k),
                    start=(k == 0),
                    stop=(k == KD - 1),
                )

            # ---- score matmuls: one PSUM tile [128, 4, 66] ----
            # pad each chunk to a full 2KB PSUM bank so the 4 accumulation
            # groups live in distinct zero regions (concurrent groups)
            sc_ps = ps_sc.tile([128, NCHUNK, 512], F32, tag="sc", name="sc_ps")
            for c in range(NCHUNK):
                for k, cT in ((0, cT0), (1, cT1)):
                    nc.tensor.matmul(
                        sc_ps[:, c, 0 : LQ + 2],
                        cT[:, 128 * c : 128 * (c + 1)],
                        rhsA[k],
                        start=(k == 0),
                        stop=False,
                    )
            # row_vec copy off the sq PSUM group (bf16, cols 64/65 stay 0)
            nc.vector.tensor_copy(row_vec[0:1, 0:LQ], sq_ps)
            for c in range(NCHUNK):
                nc.tensor.matmul(
                    sc_ps[:, c, 0 : LQ + 2], ones_bf, row_vec, start=False, stop=True
                )

            # q1 broadcast to all partitions, then to SBUF bf16
            q1_sb = singles.tile([1, LQ], BF16)
            nc.vector.tensor_copy(q1_sb, q1_ps)
            q1bc_ps = ps_misc.tile([128, LQ], F32, tag="q1bc", name="q1bc_ps")
            nc.tensor.matmul(q1bc_ps, ones_bf, q1_sb, start=True, stop=True)
            q1bc = singles.tile([128, LQ], BF16)
            nc.vector.tensor_copy(q1bc, q1bc_ps)
            # ctx1 (col 65) to SBUF fp32 for G and the H-path
            ctx1 = singles.tile([128, NCHUNK], F32)
            nc.vector.tensor_copy(ctx1, sc_ps[:, :, LQ + 1])

            # ---- ONE fused exp over scores | sc | ctx1 ----
            e_t = singles.tile([128, NCHUNK, LQ + 2], BF16)
            nc.scalar.activation(e_t, sc_ps[:, :, 0 : LQ + 2], AF.Exp)
            e_sc = e_t[:, :, LQ]  # [128, 4] = exp(sc)
            e_j = e_t[:, :, 0:LQ]  # [128, 4, 64]

            # ---- DVE reductions ----
            # H-path first: M = max_j e, u_w = exp(sc)*M, w1 = sum_c u_w
            m_t = singles.tile([128, NCHUNK], BF16)
            nc.vector.tensor_reduce(m_t, e_j, AX.X, ALU.max)
            w12 = singles.tile([128, 2], F32)
            u_w = singles.tile([128, NCHUNK], BF16)
            nc.vector.tensor_tensor_reduce(
                out=u_w,
                in0=e_sc,
                in1=m_t,
                scale=1.0,
                scalar=0.0,
                op0=ALU.mult,
                op1=ALU.add,
                accum_out=w12[:, 0:1],
            )
            uwc = singles.tile([128, NCHUNK], F32)
            nc.vector.tensor_tensor_reduce(
                out=uwc,
                in0=u_w,
                in1=ctx1,
                scale=1.0,
                scalar=0.0,
                op0=ALU.mult,
                op1=ALU.add,
                accum_out=w12[:, 1:2],
            )
            # U-path: den + num per chunk
            den = singles.tile([128, NCHUNK], F32)
            nc.vector.tensor_reduce(den, e_j, AX.X, ALU.add)
            num = singles.tile([128, NCHUNK], F32)
            prod = singles.tile([128, NCHUNK, LQ], BF16)
            for c in range(NCHUNK):
                nc.vector.tensor_tensor_reduce(
                    out=prod[:, c, :],
                    in0=e_t[:, c, 0:LQ],
                    in1=q1bc,
                    scale=1.0,
                    scalar=0.0,
                    op0=ALU.mult,
                    op1=ALU.add,
                    accum_out=num[:, c : c + 1],
                )
            rden = singles.tile([128, NCHUNK], F32)
            nc.vector.reciprocal(rden, den)

            # ---- H: partition sums via free-size-1 matmul ----
            dn_ps = ps_misc.tile([2, 1], F32, tag="dn", name="dn_ps")
            nc.tensor.matmul(dn_ps, w12, ones_col, start=True, stop=True)
            r0 = singles.tile([1, 1], F32)
            nc.vector.reciprocal(r0, dn_ps[0:1, :])
            h_sb = singles.tile([1, 1], F32)
            nc.vector.tensor_mul(h_sb, dn_ps[1:2, :], r0)
            hbc_ps = ps_misc.tile([128, 1], F32, tag="dn", name="hbc_ps")
            nc.tensor.matmul(hbc_ps, ones_f, h_sb, start=True, stop=True)
            hbc = singles.tile([128, 1], F32)
            nc.vector.tensor_copy(hbc, hbc_ps)

            # ---- G assembly on Pool (all SBUF) ----
            g_all = singles.tile([128, NCHUNK, 4], F32)
            nc.gpsimd.tensor_copy(g_all[:, :, 0], ctx1)
            nc.gpsimd.tensor_mul(g_all[:, :, 1], num, rden)
            nc.gpsimd.tensor_mul(g_all[:, :, 2], g_all[:, :, 1], ctx1)
            nc.gpsimd.tensor_scalar_mul(g_all[:, :, 3], g_all[:, :, 1], hbc)
            nc.sync.dma_start(
                out=g_d.rearrange("(c p) g -> p c g", p=128), in_=g_all
            )

    nc.finalize()
    return nc


_NC = None


def _get_nc():
    global _NC
    if _NC is None:
        _NC = build_nc()
    return _NC


def make_in_maps(context, question, mask, att_w, att_b, w_in, w_mem):
    context = np.asarray(context, np.float32)
    question = np.asarray(question, np.float32)
    mask = np.asarray(mask, np.int32)
    att_w = np.asarray(att_w, np.float32)
    att_b = np.asarray(att_b, np.float32)
    w_in = np.asarray(w_in, np.float32)
    w_mem = np.asarray(w_mem, np.float32)

    wq = att_w[0:D].reshape(KD, 128)
    # reference: w_c = att_w[:D] (context), w_q = att_w[D:2D] (question)
    wc = att_w[0:D].reshape(KD, 128)
    wq = att_w[D : 2 * D].reshape(KD, 128)
    wp = att_w[2 * D :].reshape(KD, 128)
    win = w_in.reshape(KD, 128)
    wm = w_mem.reshape(KD, 128)

    maps = []
    for b in range(B):
        ctxt = np.ascontiguousarray(context[b].T).reshape(KD, 128, LC)
        qt = np.ascontiguousarray(question[b].T).reshape(KD, 128, LQ)
        par = np.zeros((128, NPC), NPBF)
        for k in range(KD):
            par[:, PC_QT + LQ * k : PC_QT + LQ * (k + 1)] = qt[k].astype(NPBF)
            par[:, PC_WQ + k] = wq[k].astype(NPBF)
            par[:, PC_WM + k] = wm[k].astype(NPBF)
            par[:, PC_WP + k] = wp[k].astype(NPBF)
            par[:, PC_WC + k] = wc[k].astype(NPBF)
            par[:, PC_WIN + k] = win[k].astype(NPBF)
        mb = att_b[0] - NEG_BIG * (1.0 - mask[b].astype(np.float32))
        par[0, PC_MB : PC_MB + LQ] = mb.astype(NPBF)
        maps.append(
            {
                "cT0": ctxt[0].astype(NPBF),
                "cT1": ctxt[1].astype(NPBF),
                "par": par,
            }
        )
    return maps


def kernel(context, question, mask, att_w, att_b, w_in, w_mem):
    nc = _get_nc()
    in_maps = make_in_maps(context, question, mask, att_w, att_b, w_in, w_mem)
    res = run_bass_kernel_spmd(nc, in_maps, core_ids=list(range(B)))
    return np.stack([res.results[c]["G"] for c in range(B)], axis=0)


# revision 8
# speedup vs baseline: 1.3429x; 1.2844x over previous
"""Bi-attention kernel for Trainium2 (8 NeuronCores, data-parallel over batch).

Per-core computation (B=1 slice, Lc=512, Lq=64, D=256):
  score[i,j] = c_i.w_c + q_j.w_q + sum_d c[i,d] q[j,d] w_p[d] + b - 1e30*(1-mask[j])
  h = softmax_j(score);  U[i] = sum_j h[i,j] * (q_j.w_mem)
  u = softmax_i(max_j score);  H = sum_i u[i] * (c_i.w_in)
  G[i] = [ctx1[i], U[i], ctx1[i]*U[i], U[i]*H]

Layout: i (context rows) on partitions, 4 chunks of 128. All matmuls bf16
(1 cycle/row vs 4 for fp32); softmax denominators/maxima in fp32.

Scheduling (v1 CoreSim cost model):
  - params DMA on SP first (visible ~2.4us), cT k=0 on Pool, cT k=1 on SP
    second.  The Activation queue carries no DMA: the auto-inserted Exp
    table load (1283ns) runs there first and would delay it.
  - a dummy 1x1 matmul at t~300 pins pe_busy_start early so late matmuls
    hit the full 2.4GHz p-state.
  - one PSUM score tile [128, 4(chunk), 512]: each chunk padded to a full
    2KB PSUM bank so the 4 accumulation groups are concurrently open.
    Per chunk: 2 bf16 cT matmuls + 1 ones-row matmul (adds sq_j+b+mask);
    the sq/mask/bias row itself is folded in PSUM by a 3-matmul group.
  - row max runs on DVE from PSUM *concurrently* with the fused exp on
    ACT; M = exp(rmax) is a cheap [128,4] ACT op afterwards.
  - sum-reductions: den on DVE tensor_reduce; num via Pool mul + log2
    add-tree (Pool free-axis reduce is unsupported, but strided halving
    adds cost only ~210ns total); u-softmax sums via two fused
    TensorTensorReduce into w12, partition-summed by gpsimd
    partition_all_reduce (free-size-2 -> ~0ns) which also broadcasts H.
  - q1 broadcast via gpsimd partition_broadcast instead of a PE matmul.
"""

import sys

for _p in ("/opt/trn_rl_repo", "/root/.axon_site/_ro/trn_rl_repo"):
    if _p not in sys.path:
        sys.path.append(_p)

import ml_dtypes
import numpy as np

import concourse.bacc as bacc
import concourse.bass as bass
import concourse.bass_isa as bass_isa
import concourse.tile as tile
from concourse import mybir
from concourse.bass_utils import run_bass_kernel_spmd

B, LC, LQ, D = 8, 512, 64, 256
NEG_BIG = 1e30
NCHUNK = LC // 128  # 4 chunks of 128 context rows
KD = D // 128  # 2 contraction chunks
F32 = mybir.dt.float32
BF16 = mybir.dt.bfloat16
NPBF = np.dtype(ml_dtypes.bfloat16)
AF = mybir.ActivationFunctionType
ALU = mybir.AluOpType
AX = mybir.AxisListType

# params tensor column layout (bf16, packed on host)
PC_QT = 0  # cols 0:128    qT chunks: [64k : 64(k+1)] = question.T chunk k
PC_WQ = 128  # cols 128:130  w_q chunks
PC_WM = 130  # cols 130:132  w_mem chunks
PC_WP = 132  # cols 132:134  w_p chunks
PC_WC = 134  # cols 134:136  w_c chunks
PC_WIN = 136  # cols 136:138  w_in chunks
PC_MB = 138  # cols 138:202  row 0 = att_b - 1e30*(1-mask)
NPC = PC_MB + LQ


def build_nc():
    nc = bacc.Bacc("TRN2", target_bir_lowering=False, debug=False)

    ct0_d = nc.dram_tensor("cT0", [128, LC], BF16, kind="ExternalInput")
    ct1_d = nc.dram_tensor("cT1", [128, LC], BF16, kind="ExternalInput")
    par_d = nc.dram_tensor("par", [128, NPC], BF16, kind="ExternalInput")
    g_d = nc.dram_tensor("G", [LC, 4], F32, kind="ExternalOutput")

    with tile.TileContext(nc) as tc:
        with (
            tc.tile_pool(name="singles", bufs=1) as singles,
            tc.tile_pool(name="ps_sc", bufs=1, space="PSUM") as ps_sc,
            tc.tile_pool(name="ps_misc", bufs=1, space="PSUM") as ps_misc,
        ):
            # ---- first instructions per queue ----
            # Pool: tiny memset, then the cT0 DMA, then remaining memsets
            ones_bf = singles.tile([1, 128], BF16)
            nc.gpsimd.memset(ones_bf, 1.0)
            cT0 = singles.tile([128, LC], BF16)
            nc.gpsimd.dma_start(out=cT0, in_=ct0_d[:, :])
            # SP: params DMA, then cT1
            par = singles.tile([128, NPC], BF16)
            nc.sync.dma_start(out=par, in_=par_d[:, :])
            cT1 = singles.tile([128, LC], BF16)
            nc.sync.dma_start(out=cT1, in_=ct1_d[:, :])
            # PE: dummy matmul to pin pe_busy_start early (p-state ramp)
            dmy_ps = ps_misc.tile([1, 1], F32, tag="dmy", name="dmy_ps")
            nc.tensor.matmul(
                dmy_ps, ones_bf[0:1, 0:1], ones_bf[0:1, 0:1], start=True, stop=True
            )

            ones_f = singles.tile([1, 128], F32)
            nc.gpsimd.memset(ones_f, 1.0)
            row_vec = singles.tile([1, LQ + 2], BF16)
            nc.gpsimd.memset(row_vec, 0.0)
            warm = singles.tile([1, 1], F32)
            nc.scalar.activation(warm, ones_f[0:1, 0:1], AF.Exp)

            def qt(k):
                return par[:, PC_QT + LQ * k : PC_QT + LQ * (k + 1)]

            # rhsA_k [128, 66]: cols 0:64 = w_p * qT, col 64 = w_c, col 65 = w_in
            rhsA = []
            for k, eng in ((0, nc.gpsimd), (1, nc.vector)):
                r = singles.tile([128, LQ + 2], BF16, tag=f"rhsA{k}", name=f"rhsA{k}")
                wp_f = singles.tile([128, 1], F32, tag=f"wpf{k}", name=f"wpf{k}")
                eng.tensor_copy(wp_f, par[:, PC_WP + k : PC_WP + k + 1])
                eng.tensor_scalar_mul(r[:, 0:LQ], qt(k), wp_f)
                eng.tensor_copy(r[:, LQ : LQ + 1], par[:, PC_WC + k : PC_WC + k + 1])
                eng.tensor_copy(
                    r[:, LQ + 1 : LQ + 2], par[:, PC_WIN + k : PC_WIN + k + 1]
                )
                rhsA.append(r)

            # ---- sq group: sq_j + b - 1e30*(1-mask_j), folded in PSUM ----
            sq_ps = ps_misc.tile([1, LQ], F32, tag="sq", name="sq_ps")
            for k in range(KD):
                nc.tensor.matmul(
                    sq_ps,
                    par[:, PC_WQ + k : PC_WQ + k + 1],
                    qt(k),
                    start=(k == 0),
                    stop=False,
                )
            nc.tensor.matmul(
                sq_ps,
                ones_bf[0:1, 0:1],
                par[0:1, PC_MB : PC_MB + LQ],
                start=False,
                stop=True,
            )
            # q1 group: q1_j = q_j . w_mem
            q1_ps = ps_misc.tile([1, LQ], F32, tag="q1", name="q1_ps")
            for k in range(KD):
                nc.tensor.matmul(
                    q1_ps,
                    par[:, PC_WM + k : PC_WM + k + 1],
                    qt(k),
                    start=(k == 0),
                    stop=(k == KD - 1),
                )

            # ---- score matmuls ----
            # each chunk padded to a full 2KB PSUM bank: 4 concurrently open
            # accumulation groups in 4 distinct zero regions
            sc_ps = ps_sc.tile([128, NCHUNK, 512], F32, tag="sc", name="sc_ps")
            for k, cT in ((0, cT0), (1, cT1)):
                for c in range(NCHUNK):
                    nc.tensor.matmul(
                        sc_ps[:, c, 0 : LQ + 2],
                        cT[:, 128 * c : 128 * (c + 1)],
                        rhsA[k],
                        start=(k == 0),
                        stop=False,
                    )
            # row_vec copy off the sq PSUM group (bf16, cols 64/65 stay 0)
            nc.vector.tensor_copy(row_vec[0:1, 0:LQ], sq_ps)
            for c in range(NCHUNK):
                nc.tensor.matmul(
                    sc_ps[:, c, 0 : LQ + 2], ones_bf, row_vec, start=False, stop=True
                )

            # q1 row to SBUF, broadcast to all partitions on Pool
            q1_sb = singles.tile([1, LQ], BF16)
            nc.vector.tensor_copy(q1_sb, q1_ps)
            q1bc = singles.tile([128, LQ], BF16)
            nc.gpsimd.partition_broadcast(q1bc, q1_sb)

            # ---- row max from PSUM, concurrent with exp on ACT ----
            rmax = singles.tile([128, NCHUNK], F32)
            nc.vector.tensor_reduce(rmax, sc_ps[:, :, 0:LQ], AX.X, ALU.max)
            # ctx1 (col 65) to SBUF fp32
            ctx1 = singles.tile([128, NCHUNK], F32)
            nc.vector.tensor_copy(ctx1, sc_ps[:, :, LQ + 1])

            # ---- fused exp over scores | sc (col 65 junk but harmless) ----
            e_t = singles.tile([128, NCHUNK, LQ + 2], BF16)
            nc.scalar.activation(e_t, sc_ps[:, :, 0 : LQ + 2], AF.Exp)
            e_sc = e_t[:, :, LQ]  # [128, 4] = exp(sc)
            # M = exp(max_j score) on ACT right after the main exp
            m_t = singles.tile([128, NCHUNK], BF16)
            nc.scalar.activation(m_t, rmax, AF.Exp)

            # ---- u-softmax sums: u_w = exp(sc)*M, w1 = sum u_w, w2 = sum u_w*ctx1
            w12 = singles.tile([128, 2], F32)
            u_w = singles.tile([128, NCHUNK], BF16)
            nc.vector.tensor_tensor_reduce(
                out=u_w,
                in0=e_sc,
                in1=m_t,
                scale=1.0,
                scalar=0.0,
                op0=ALU.mult,
                op1=ALU.add,
                accum_out=w12[:, 0:1],
            )
            uwc = singles.tile([128, NCHUNK], F32)
            nc.vector.tensor_tensor_reduce(
                out=uwc,
                in0=u_w,
                in1=ctx1,
                scale=1.0,
                scalar=0.0,
                op0=ALU.mult,
                op1=ALU.add,
                accum_out=w12[:, 1:2],
            )

            # ---- den on DVE; num via Pool mul + log2 add-tree ----
            den = singles.tile([128, NCHUNK], F32)
            nc.vector.tensor_reduce(den, e_t[:, :, 0:LQ], AX.X, ALU.add)
            rden = singles.tile([128, NCHUNK], F32)
            nc.vector.reciprocal(rden, den)

            prod = singles.tile([128, NCHUNK, LQ], BF16)
            for c in range(NCHUNK):
                nc.gpsimd.tensor_mul(prod[:, c, :], e_t[:, c, 0:LQ], q1bc)
            ntree = singles.tile([128, NCHUNK, 32], F32)
            nc.gpsimd.tensor_add(ntree, prod[:, :, 0:32], prod[:, :, 32:64])
            w = 16
            while w >= 1:
                nc.gpsimd.tensor_add(
                    ntree[:, :, 0:w], ntree[:, :, 0:w], ntree[:, :, w : 2 * w]
                )
                w //= 2
            num = ntree[:, :, 0]  # [128, 4]

            # ---- H via partition_all_reduce (result on every partition) ----
            pr = singles.tile([128, 2], F32)
            nc.gpsimd.partition_all_reduce(
                pr, w12, channels=128, reduce_op=bass_isa.ReduceOp.add
            )
            rs1 = singles.tile([128, 1], F32)
            nc.vector.reciprocal(rs1, pr[:, 0:1])
            h_t = singles.tile([128, 1], F32)
            nc.gpsimd.tensor_mul(h_t, pr[:, 1:2], rs1)

            # ---- G assembly on Pool (all SBUF) ----
            g_all = singles.tile([128, NCHUNK, 4], F32)
            nc.gpsimd.tensor_copy(g_all[:, :, 0], ctx1)
            nc.gpsimd.tensor_mul(g_all[:, :, 1], num, rden)
            nc.gpsimd.tensor_mul(g_all[:, :, 2], g_all[:, :, 1], ctx1)
            nc.gpsimd.tensor_scalar_mul(g_all[:, :, 3], g_all[:, :, 1], h_t)
            nc.sync.dma_start(
                out=g_d.rearrange("(c p) g -> p c g", p=128), in_=g_all
            )

    nc.finalize()
    return nc


_NC = None


def _get_nc():
    global _NC
    if _NC is None:
        _NC = build_nc()
    return _NC


def make_in_maps(context, question, mask, att_w, att_b, w_in, w_mem):
    context = np.asarray(context, np.float32)
    question = np.asarray(question, np.float32)
    mask = np.asarray(mask, np.int32)
    att_w = np.asarray(att_w, np.float32)
    att_b = np.asarray(att_b, np.float32)
    w_in = np.asarray(w_in, np.float32)
    w_mem = np.asarray(w_mem, np.float32)

    # reference: w_c = att_w[:D] (context), w_q = att_w[D:2D] (question)
    wc = att_w[0:D].reshape(KD, 128)
    wq = att_w[D : 2 * D].reshape(KD, 128)
    wp = att_w[2 * D :].reshape(KD, 128)
    win = w_in.reshape(KD, 128)
    wm = w_mem.reshape(KD, 128)

    maps = []
    for b in range(B):
        ctxt = np.ascontiguousarray(context[b].T).reshape(KD, 128, LC)
        qt = np.ascontiguousarray(question[b].T).reshape(KD, 128, LQ)
        par = np.zeros((128, NPC), NPBF)
        for k in range(KD):
            par[:, PC_QT + LQ * k : PC_QT + LQ * (k + 1)] = qt[k].astype(NPBF)
            par[:, PC_WQ + k] = wq[k].astype(NPBF)
            par[:, PC_WM + k] = wm[k].astype(NPBF)
            par[:, PC_WP + k] = wp[k].astype(NPBF)
            par[:, PC_WC + k] = wc[k].astype(NPBF)
            par[:, PC_WIN + k] = win[k].astype(NPBF)
        mb = att_b[0] - NEG_BIG * (1.0 - mask[b].astype(np.float32))
        par[0, PC_MB : PC_MB + LQ] = mb.astype(NPBF)
        maps.append(
            {
                "cT0": ctxt[0].astype(NPBF),
                "cT1": ctxt[1].astype(NPBF),
                "par": par,
            }
        )
    return maps


def kernel(context, question, mask, att_w, att_b, w_in, w_mem):
    nc = _get_nc()
    in_maps = make_in_maps(context, question, mask, att_w, att_b, w_in, w_mem)
    res = run_bass_kernel_spmd(nc, in_maps, core_ids=list(range(B)))
    return np.stack([res.results[c]["G"] for c in range(B)], axis=0)
